# revision 45
# baseline (speedup 1.0000x reference)
"""Trainium2 Bass kernel for causal MultiHeadAttention.

Problem: B=4, S=2048, H=16, D=64, DM=1024, fp32 I/O.
  qkv = x @ w_qkv ; causal softmax attention per head ; out = attn @ w_out

Sharding (8 cores): 4-way batch x 2-way heads. Core c handles batch c//2 and
heads (c%2)*8 .. +8. Each core computes a partial out-projection (its 512
attention channels x full w_out row-slice); the host sums the two head-half
partials per batch while unsharding.

Per-core dataflow (all bf16 matmul inputs, fp32 PSUM):
  xt = x[b].T (host)                                  [1024, 2048]
  qT,kT = (w_qk_slice).T-major proj:  lhsT=w tiles, rhs=xt   -> [512ch, 2048]
  v    = row-major proj:             lhsT=xt tiles, rhs=w_v  -> [2048, 512ch]
  scoresT[ki,qi] = kT.T @ qT   (K=64, two heads row-packed in PE array)
  probsT = exp(scoresT)  (no max pass: |scores| <= ~7)
  diagonal blocks: probsT *= tri01 (bf16 0/1 upper-tri mask, DVE 2x mode)
  outT[d,qi]  = v.T-contract: lhsT=[v[ki,d]|ones] (M=65; row 64 = softmax denom)
  normalize (fused with PSUM eviction):
    recip_approx(denom row) -> DMA shift to partition 0 -> gpsimd
    partition_broadcast -> tensor_mul(oT_dst, psum_out[0:64], bcast)
  partial_out = out_headsT.T @ w_out_slice  (row-major psum -> sbuf -> HBM)

PE filler: V row-tiles + next pair's Q/K projections are interleaved into
the attention block stream of pairs 0-2; the out-projection row-tiles are
interleaved into pair 3 (gated per query-supertile) so the tensor engine
stays busy during exp waits everywhere.
"""

import numpy as np

B, S, H, D = 4, 2048, 16, 64
DM = H * D          # 1024
NCORES = 8
HPC = H // 2        # 8 heads per core
CQ = HPC * D        # 512 channels per core

_PROG_CACHE = {}

# normalize implementation: "fast" = recip_approx from PSUM + gpsimd
# partition_broadcast + fused eviction multiply; "bcast" = baseline recip
# chain but gpsimd broadcast; "baseline" = proven recip + bc-matmul chain
NORM_MODE = "fast2"


def build_program(rep_qkv=1, rep_attn=1, rep_oproj=1, rep_all=1):
    import concourse.mybir as mybir
    import concourse.tile as tile
    from concourse import bacc

    dt = mybir.dt
    f32 = dt.float32
    bf16 = dt.bfloat16
    AF = mybir.ActivationFunctionType

    nc = bacc.Bacc(None)
    xt = nc.declare_dram_parameter("xt", [DM, S], bf16, isOutput=False)
    wqk = nc.declare_dram_parameter("wqk", [DM, 2 * CQ], bf16, isOutput=False)
    wv = nc.declare_dram_parameter("wv", [DM, CQ], bf16, isOutput=False)
    wo = nc.declare_dram_parameter("wo", [CQ, DM], bf16, isOutput=False)
    tri = nc.declare_dram_parameter("tri", [128, 128], bf16, isOutput=False)
    out = nc.declare_dram_parameter("out", [S, DM], f32, isOutput=True)

    KT = DM // 128      # 8 contraction tiles over model dim
    NRT = S // 128      # 16 row tiles over sequence
    NRC = S // 512      # 4 row chunks over sequence
    NP = HPC // 2       # 4 head pairs per core
    NST = S // 512      # 4 query supertiles

    with tile.TileContext(nc) as tc:
        with (
            tc.tile_pool(name="persist", bufs=1) as pp,
            tc.tile_pool(name="probs", bufs=6) as probsp,
            tc.tile_pool(name="recip", bufs=3) as recipp,
            tc.tile_pool(name="hip", bufs=2) as hip,
            tc.tile_pool(name="ostage", bufs=3) as ostagep,
            tc.tile_pool(name="psmm", bufs=2, space="PSUM") as psmm,
            tc.tile_pool(name="pssc", bufs=2, space="PSUM") as pssc,
            tc.tile_pool(name="psout", bufs=2, space="PSUM") as psout,
        ):
            # ---- load inputs to SBUF ----
            xt_sb = []
            wqk_sb = []
            wv_sb = []
            for i in range(KT):
                t = pp.tile([128, S], bf16, tag=f"xt{i}", name=f"xt{i}")
                nc.sync.dma_start(out=t[:], in_=xt[128 * i:128 * (i + 1), :])
                xt_sb.append(t)
                t = pp.tile([128, 2 * CQ], bf16, tag=f"wqk{i}", name=f"wqk{i}")
                nc.scalar.dma_start(out=t[:], in_=wqk[128 * i:128 * (i + 1), :])
                wqk_sb.append(t)
                t = pp.tile([128, CQ], bf16, tag=f"wv{i}", name=f"wv{i}")
                nc.scalar.dma_start(out=t[:], in_=wv[128 * i:128 * (i + 1), :])
                wv_sb.append(t)
            wo_sb = []
            for c in range(CQ // 128):
                t = pp.tile([128, DM], bf16, tag=f"wo{c}", name=f"wo{c}")
                nc.scalar.dma_start(out=t[:], in_=wo[128 * c:128 * (c + 1), :])
                wo_sb.append(t)
            tri_sb = pp.tile([128, 128], bf16, tag="tri", name="tri")
            nc.sync.dma_start(out=tri_sb[:], in_=tri[:, :])
            ones64_sb = pp.tile([1, 64], bf16, tag="ones64", name="ones64")
            nc.vector.memset(ones64_sb[:, :], 1.0)

            # persistent activation tensors; v tiles hold 65 cols per head
            # (64 v-channels + a ones column so PV also accumulates the
            # softmax denominator into output row 64)
            qT = [pp.tile([128, S], bf16, tag=f"qT{p}", name=f"qT{p}") for p in range(NP)]
            kT = [pp.tile([128, S], bf16, tag=f"kT{p}", name=f"kT{p}") for p in range(NP)]
            v_rm = [pp.tile([128, HPC * 65], bf16, tag=f"v{rt}", name=f"v{rt}") for rt in range(NRT)]
            oT = [pp.tile([128, S], bf16, tag=f"oT{p}", name=f"oT{p}") for p in range(NP)]

            def v_group_steps(rt):
                # V-projection psum group for row-tile rt, as one micro-step
                # (single matmul) per list entry so fillers can interleave
                # at ~200ns granularity between attention blocks
                state = {}

                def step(kt):
                    if kt == 0:
                        v_view = v_rm[rt].rearrange("p (h c) -> p h c", c=65)
                        nc.vector.memset(v_view[:, :, 64:65], 1.0)
                        state["ps"] = psmm.tile([128, 512], f32, tag="mm", name="mm")
                    nc.tensor.matmul(
                        state["ps"][:],
                        lhsT=xt_sb[kt][:, 128 * rt:128 * (rt + 1)],
                        rhs=wv_sb[kt][:],
                        start=(kt == 0),
                        stop=(kt == KT - 1),
                    )
                    if kt == KT - 1:
                        v_view = v_rm[rt].rearrange("p (h c) -> p h c", c=65)
                        nc.vector.tensor_copy(
                            v_view[:, :, 0:64],
                            state["ps"].rearrange("p (h c) -> p h c", c=64),
                        )

                return [lambda a=kt: step(a) for kt in range(KT)]

            def v_group(rt):
                for s in v_group_steps(rt):
                    s()

            def oproj_steps(rt):
                # out-projection row tile as 8 micro-steps + HBM store
                state = {}

                def step(i):
                    o2, c = i // 4, i % 4
                    if c == 0:
                        if o2 == 0:
                            state["st"] = ostagep.tile(
                                [128, 1024], f32, tag="ostage", name="ostage"
                            )
                        state["ps"] = psmm.tile([128, 512], f32, tag="mm", name="mm")
                    nc.tensor.matmul(
                        state["ps"][:],
                        lhsT=oT[c][:, 128 * rt:128 * (rt + 1)],
                        rhs=wo_sb[c][:, 512 * o2:512 * (o2 + 1)],
                        start=(c == 0),
                        stop=(c == 3),
                    )
                    if c == 3:
                        nc.vector.tensor_copy(
                            state["st"][:, 512 * o2:512 * (o2 + 1)], state["ps"][:]
                        )
                        if o2 == 1:
                            # store on the (idle) gpsimd queue so the big
                            # 512KB transfers never queue ahead of the
                            # latency-critical hi/normalize DMAs on sync
                            nc.gpsimd.dma_start(
                                out=out[128 * rt:128 * (rt + 1), :],
                                in_=state["st"][:],
                            )

                return [lambda a=i: step(a) for i in range(8)]

            def oproj_rt(rt):
                for s in oproj_steps(rt):
                    s()

            for _arep in range(rep_all):

             def qk_group_steps(p, ct, rc):
                 # one Q/K projection psum group as 8 micro-steps + evict
                 dst = qT[p] if ct < NP else kT[p]
                 state = {}

                 def step(kt):
                     if kt == 0:
                         state["ps"] = psmm.tile([128, 512], f32, tag="mm", name="mm")
                     nc.tensor.matmul(
                         state["ps"][:],
                         lhsT=wqk_sb[kt][:, 128 * ct:128 * (ct + 1)],
                         rhs=xt_sb[kt][:, 512 * rc:512 * (rc + 1)],
                         start=(kt == 0),
                         stop=(kt == KT - 1),
                     )
                     if kt == KT - 1:
                         nc.vector.tensor_copy(
                             dst[:, 512 * rc:512 * (rc + 1)], state["ps"][:]
                         )

                 return [lambda a=kt: step(a) for kt in range(KT)]

             def qk_steps(p):
                 return [
                     s
                     for ct in (p, NP + p)
                     for rc in range(NRC)
                     for s in qk_group_steps(p, ct, rc)
                 ]

             qk_groups = lambda p: [
                 (lambda a=p, b=ct, c=rc: [s() for s in qk_group_steps(a, b, c)])
                 for ct in (p, NP + p) for rc in range(NRC)
             ]

             def prologue():
                 for g in qk_groups(0):
                     g()
                 # V row-tiles 0..3 are needed by attention(0) st=0; the
                 # rest are interleave filler inside attention(0)
                 for rt in range(4):
                     v_group(rt)

             if _arep == 0:
                 prologue()

             def make_fill(p):
                 # per-st filler micro-step lists (one PE matmul each):
                 # sprinkled between every attention block so the PE always
                 # has work covering the exp latency. v_rm[rt] must be
                 # emitted before the st that streams it; qk(p+1) spreads
                 # over pair p; oproj(rt) spreads over pair 3 gated on the
                 # st that produced its oT rows.
                 fill = {0: [], 1: [], 2: [], 3: []}
                 if p == 0:
                     fill[0] = [s for rt in range(4, 8) for s in v_group_steps(rt)]
                     fill[1] = [s for rt in range(8, 16) for s in v_group_steps(rt)]
                     qs = qk_steps(1)
                     fill[2] = qs[: len(qs) // 2]
                     fill[3] = qs[len(qs) // 2:]
                 elif p < 3:
                     qs = qk_steps(p + 1)
                     fill[0] = qs[:8]
                     fill[1] = qs[8:24]
                     fill[2] = qs[24:44]
                     fill[3] = qs[44:]
                 else:
                     fill[1] = [s for rt in range(0, 4) for s in oproj_steps(rt)]
                     fill[2] = [s for rt in range(4, 8) for s in oproj_steps(rt)]
                     fill[3] = [s for rt in range(8, 12) for s in oproj_steps(rt)]
                 return fill

             pair_jobs = [(p, make_fill(p)) for p in range(NP)]
             for _extra in range(rep_attn - 1):
                 pair_jobs += [(p, {0: [], 1: [], 2: [], 3: []}) for p in range(NP)]

             for p, fill in pair_jobs:
                 hi_sb = hip.tile([64, S], bf16, tag="hi", name="hi")
                 for st in range(NST):
                     out_ps = [
                         psout.tile([65, 512], f32, tag="o", name="o")
                         for _ in range(2)
                     ]
                     nkb = 4 * st + 4
                     fillers = fill[st]
                     nfill = len(fillers)
                     fill_pos = 0

                     def pv_block(kb, pr):
                         r = kb - 4 * st
                         qi0 = 128 * r if r > 0 else 0
                         for hh in range(2):
                             base = 512 * hh
                             h = 2 * p + hh
                             nc.tensor.matmul(
                                 out_ps[hh][:, qi0:512],
                                 lhsT=v_rm[kb][:, 65 * h:65 * h + 65],
                                 rhs=pr[:, base + qi0:base + 512],
                                 start=(kb == 0),
                                 stop=(kb == nkb - 1),
                             )

                     # inner loop runs two blocks per unit, software-
                     # pipelined one unit deep: both blocks' scores+exp are
                     # emitted back-to-back (a 2-exp ACT backlog), then
                     # filler matmuls, then the previous unit's PVs — the
                     # PE streams projection work while ACT drains exps
                     # without the fillers postponing the score feed.
                     def sc_exp_block(kb):
                         r = kb - 4 * st  # >=0: diagonal block, tri-masked
                         qi0 = 128 * r if r > 0 else 0
                         sc = pssc.tile([128, 1024], f32, tag="sc", name="sc")
                         for hh in range(2):
                             base, lo = 512 * hh, 64 * hh
                             nc.tensor.matmul(
                                 sc[:, base + qi0:base + 512],
                                 lhsT=kT[p][lo:lo + 64, 128 * kb:128 * (kb + 1)],
                                 rhs=qT[p][lo:lo + 64, 512 * st + qi0:512 * (st + 1)],
                                 start=True,
                                 stop=True,
                                 tile_position=(lo, 0),
                             )
                         pr = probsp.tile([128, 1024], bf16, tag="pr", name="pr")
                         if qi0 == 0:
                             nc.scalar.activation(pr[:], sc[:], AF.Exp)
                         else:
                             pr_v = pr.rearrange("p (h q) -> p h q", h=2)
                             sc_v = sc.rearrange("p (h q) -> p h q", h=2)
                             nc.scalar.activation(
                                 pr_v[:, :, qi0:512], sc_v[:, :, qi0:512], AF.Exp
                             )
                         if r >= 0:
                             # causal zeroing of the diagonal stripe: probs
                             # *= upper-tri 0/1 mask (both heads, one DVE op)
                             pr_v = pr.rearrange("p (h q) -> p h q", h=2)
                             nc.vector.tensor_mul(
                                 pr_v[:, :, qi0:qi0 + 128],
                                 pr_v[:, :, qi0:qi0 + 128],
                                 tri_sb[:, None, :].broadcast_to([128, 2, 128]),
                             )
                         return (kb, pr)

                     prev_unit = []
                     for kb2 in range(0, nkb, 2):
                         cur = [sc_exp_block(kb2), sc_exp_block(kb2 + 1)]
                         target = nfill * (kb2 + 2) // nkb
                         while fill_pos < target:
                             fillers[fill_pos]()
                             fill_pos += 1
                         for item in prev_unit:
                             pv_block(*item)
                         prev_unit = cur
                     for item in prev_unit:
                         pv_block(*item)
                     # normalize: oT_dst = out_ps[0:64] * recip(denom row 64)
                     # with the recip row broadcast across partitions.
                     # hh=1 first so the partition-shift DMA issues while
                     # hh=0's recip/mul still run.
                     if NORM_MODE == "fast2":
                         # evict both [65,512] PSUM tiles immediately (data
                         # + denom row together — frees psout banks fast),
                         # then DMA-reshape the denom rows to [128,8] so
                         # the iterative reciprocal is ~100ns (cost scales
                         # with free size), reshape back, one gpsimd
                         # broadcast, and all-SBUF normalize multiplies.
                         o_sbs = []
                         for hh in (1, 0):
                             o_sb = recipp.tile(
                                 [65, 512], f32, tag=f"osb{hh}", name=f"osb{hh}"
                             )
                             nc.vector.tensor_copy(o_sb[:, :], out_ps[hh][:, :])
                             o_sbs.append((hh, o_sb))
                         d128 = recipp.tile([128, 8], f32, tag="d128", name="d128")
                         d128b = recipp.tile([128, 8], bf16, tag="d128b", name="d128b")
                         rrow = recipp.tile([1, 1024], bf16, tag="rc0", name="rc0")
                         bcast = recipp.tile([64, 1024], bf16, tag="bc", name="bc")
                         for hh, o_sb in o_sbs:
                             nc.sync.dma_start(
                                 out=d128[:, 4 * hh:4 * hh + 4],
                                 in_=o_sb[64:65, :],
                             )
                         with nc.allow_low_precision(reason="bf16 softmax denom"):
                             nc.vector.reciprocal(d128b[:, :], d128[:, :])
                         for hh, o_sb in o_sbs:
                             nc.sync.dma_start(
                                 out=rrow[0:1, 512 * hh:512 * (hh + 1)],
                                 in_=d128b[:, 4 * hh:4 * hh + 4],
                             )
                         nc.gpsimd.partition_broadcast(bcast[:, :], rrow[0:1, :])
                         for hh, o_sb in o_sbs:
                             dst = (
                                 oT[p][0:64, 512 * st:512 * (st + 1)]
                                 if hh == 0
                                 else hi_sb[:, 512 * st:512 * (st + 1)]
                             )
                             nc.vector.tensor_mul(
                                 dst,
                                 o_sb[0:64, :],
                                 bcast[:, 512 * hh:512 * (hh + 1)],
                             )
                             if hh == 1:
                                 # partition-shift DMA for the odd head
                                 nc.sync.dma_start(
                                     out=oT[p][64:128, 512 * st:512 * (st + 1)],
                                     in_=hi_sb[:, 512 * st:512 * (st + 1)],
                                 )
                     for hh in (1, 0) if NORM_MODE != "fast2" else ():
                         dst = (
                             oT[p][0:64, 512 * st:512 * (st + 1)]
                             if hh == 0
                             else hi_sb[:, 512 * st:512 * (st + 1)]
                         )
                         if True:
                             o_sb = recipp.tile([65, 512], f32, tag="osb", name="osb")
                             nc.vector.tensor_copy(o_sb[:, :], out_ps[hh][:, :])
                             rc_sb = recipp.tile([65, 512], bf16, tag="rc", name="rc")
                             rc0_sb = recipp.tile([1, 512], bf16, tag="rc0", name="rc0")
                             with nc.allow_low_precision(reason="bf16 softmax denom"):
                                 nc.vector.reciprocal(rc_sb[64:65, :], o_sb[64:65, :])
                             nc.sync.dma_start(out=rc0_sb[0:1, :], in_=rc_sb[64:65, :])
                             if NORM_MODE == "bcast":
                                 bcast = recipp.tile([64, 512], bf16, tag="bc", name="bc")
                                 nc.gpsimd.partition_broadcast(
                                     bcast[:, :], rc0_sb[0:1, :]
                                 )
                                 nc.vector.tensor_mul(
                                     dst, o_sb[0:64, :], bcast[:, :]
                                 )
                             else:
                                 bc_ps = psmm.tile([64, 512], f32, tag="mm", name="mm")
                                 nc.tensor.matmul(
                                     bc_ps[:, :],
                                     lhsT=ones64_sb[0:1, :],
                                     rhs=rc0_sb[0:1, :],
                                     start=True,
                                     stop=True,
                                 )
                                 nc.vector.tensor_mul(
                                     dst, o_sb[0:64, :], bc_ps[:, :]
                                 )
                         if hh == 1:
                             # partition-shift DMA for the odd head
                             nc.sync.dma_start(
                                 out=oT[p][64:128, 512 * st:512 * (st + 1)],
                                 in_=hi_sb[:, 512 * st:512 * (st + 1)],
                             )

             # phase-doubling knobs for differential HW attribution
             for _extra in range(rep_qkv - 1):
                 for p2 in range(NP):
                     for g in qk_groups(p2):
                         g()
                 for rt2 in range(NRT):
                     v_group(rt2)
             for _extra in range(rep_oproj - 1):
                 for rt2 in range(NRT):
                     oproj_rt(rt2)
             # software-pipeline across reps: the next rep's projection
             # prologue has no dependence on this rep's last normalize
             # chain, so emit it BEFORE the out-projection tail — the PE
             # FIFO then fills the tail's dependency wait with real work.
             if _arep + 1 < rep_all:
                 prologue()
             # ---- out-projection tail: last four row tiles ----
             for rt in range(12, 16):
                 oproj_rt(rt)
    nc.finalize()
    return nc


def get_program():
    if "nc" not in _PROG_CACHE:
        _PROG_CACHE["nc"] = build_program()
    return _PROG_CACHE["nc"]


def make_in_maps(x, w_qkv, w_out):
    import ml_dtypes

    bf = ml_dtypes.bfloat16
    x = np.asarray(x, dtype=np.float32)
    w_qkv = np.asarray(w_qkv, dtype=np.float32)
    w_out = np.asarray(w_out, dtype=np.float32)
    scale = float(D) ** -0.5
    # upper-tri (incl diagonal) 0/1 mask: element [p, j] of a diagonal
    # 128x128 stripe survives iff key p <= query j
    p_idx = np.arange(128)[:, None]
    j_idx = np.arange(128)[None, :]
    tri = (p_idx <= j_idx).astype(bf)
    in_maps = []
    for c in range(NCORES):
        b, hh = c // 2, c % 2
        q0 = CQ * hh
        wq = (w_qkv[:, q0:q0 + CQ] * scale).astype(bf)
        wk = w_qkv[:, DM + q0:DM + q0 + CQ].astype(bf)
        in_maps.append(
            {
                "xt": np.ascontiguousarray(x[b].T).astype(bf),
                "wqk": np.concatenate([wq, wk], axis=1),
                "wv": w_qkv[:, 2 * DM + q0:2 * DM + q0 + CQ].astype(bf),
                "wo": w_out[q0:q0 + CQ, :].astype(bf),
                "tri": tri,
            }
        )
    return in_maps


def gather(results):
    outs = [np.asarray(results[c]["out"], dtype=np.float32) for c in range(NCORES)]
    return np.stack([outs[2 * b] + outs[2 * b + 1] for b in range(B)], axis=0)


def kernel(x, w_qkv, w_out):
    from concourse.bass_utils import run_bass_kernel_spmd

    nc = get_program()
    in_maps = make_in_maps(x, w_qkv, w_out)
    res = run_bass_kernel_spmd(nc, in_maps, list(range(NCORES)))
    return gather(res.results)


# revision 48
# speedup vs baseline: 1.0652x; 1.0652x over previous
"""Trainium2 Bass kernel for causal MultiHeadAttention.

Problem: B=4, S=2048, H=16, D=64, DM=1024, fp32 I/O.
  qkv = x @ w_qkv ; causal softmax attention per head ; out = attn @ w_out

Sharding (8 cores): 4-way batch x 2-way heads. Core c handles batch c//2 and
heads (c%2)*8 .. +8. Each core computes a partial out-projection (its 512
attention channels x full w_out row-slice); the host sums the two head-half
partials per batch while unsharding.

Per-core dataflow (all bf16 matmul inputs, fp32 PSUM):
  xt = x[b].T (host)                                  [1024, 2048]
  qT,kT = (w_qk_slice).T-major proj:  lhsT=w tiles, rhs=xt   -> [512ch, 2048]
  v    = row-major proj:             lhsT=xt tiles, rhs=w_v  -> [2048, 512ch]
  scoresT[ki,qi] = kT.T @ qT   (K=64, two heads row-packed in PE array)
  probsT = exp(scoresT)  (no max pass: |scores| <= ~7)
  diagonal blocks: probsT *= tri01 (bf16 0/1 upper-tri mask, DVE 2x mode)
  outT[d,qi]  = v.T-contract: lhsT=[v[ki,d]|ones] (M=65; row 64 = softmax denom)
  normalize (fused with PSUM eviction):
    recip_approx(denom row) -> DMA shift to partition 0 -> gpsimd
    partition_broadcast -> tensor_mul(oT_dst, psum_out[0:64], bcast)
  partial_out = out_headsT.T @ w_out_slice  (row-major psum -> sbuf -> HBM)

PE filler: V row-tiles + next pair's Q/K projections are interleaved into
the attention block stream of pairs 0-2; the out-projection row-tiles are
interleaved into pair 3 (gated per query-supertile) so the tensor engine
stays busy during exp waits everywhere.
"""

import numpy as np

B, S, H, D = 4, 2048, 16, 64
DM = H * D          # 1024
NCORES = 8
HPC = H // 2        # 8 heads per core
CQ = HPC * D        # 512 channels per core

_PROG_CACHE = {}

# normalize implementation: "fast" = recip_approx from PSUM + gpsimd
# partition_broadcast + fused eviction multiply; "bcast" = baseline recip
# chain but gpsimd broadcast; "baseline" = proven recip + bc-matmul chain
NORM_MODE = "fast2"


def build_program(rep_qkv=1, rep_attn=1, rep_oproj=1, rep_all=1):
    import concourse.mybir as mybir
    import concourse.tile as tile
    from concourse import bacc

    dt = mybir.dt
    f32 = dt.float32
    bf16 = dt.bfloat16
    AF = mybir.ActivationFunctionType

    nc = bacc.Bacc(None)
    xt = nc.declare_dram_parameter("xt", [DM, S], bf16, isOutput=False)
    wqk = nc.declare_dram_parameter("wqk", [DM, 2 * CQ], bf16, isOutput=False)
    wv = nc.declare_dram_parameter("wv", [DM, CQ], bf16, isOutput=False)
    wo = nc.declare_dram_parameter("wo", [CQ, DM], bf16, isOutput=False)
    tri = nc.declare_dram_parameter("tri", [128, 128], bf16, isOutput=False)
    out = nc.declare_dram_parameter("out", [S, DM], f32, isOutput=True)

    KT = DM // 128      # 8 contraction tiles over model dim
    NRT = S // 128      # 16 row tiles over sequence
    NRC = S // 512      # 4 row chunks over sequence
    NP = HPC // 2       # 4 head pairs per core
    NST = S // 512      # 4 query supertiles

    with tile.TileContext(nc) as tc:
        with (
            tc.tile_pool(name="persist", bufs=1) as pp,
            tc.tile_pool(name="probs", bufs=6) as probsp,
            tc.tile_pool(name="recip", bufs=3) as recipp,
            tc.tile_pool(name="hip", bufs=2) as hip,
            tc.tile_pool(name="ostage", bufs=3) as ostagep,
            tc.tile_pool(name="psmm", bufs=2, space="PSUM") as psmm,
            tc.tile_pool(name="pssc", bufs=2, space="PSUM") as pssc,
            tc.tile_pool(name="psout", bufs=2, space="PSUM") as psout,
        ):
            # ---- load inputs to SBUF ----
            xt_sb = []
            wqk_sb = []
            wv_sb = []
            for i in range(KT):
                t = pp.tile([128, S], bf16, tag=f"xt{i}", name=f"xt{i}")
                nc.sync.dma_start(out=t[:], in_=xt[128 * i:128 * (i + 1), :])
                xt_sb.append(t)
                t = pp.tile([128, 2 * CQ], bf16, tag=f"wqk{i}", name=f"wqk{i}")
                nc.scalar.dma_start(out=t[:], in_=wqk[128 * i:128 * (i + 1), :])
                wqk_sb.append(t)
                t = pp.tile([128, CQ], bf16, tag=f"wv{i}", name=f"wv{i}")
                nc.scalar.dma_start(out=t[:], in_=wv[128 * i:128 * (i + 1), :])
                wv_sb.append(t)
            wo_sb = []
            for c in range(CQ // 128):
                t = pp.tile([128, DM], bf16, tag=f"wo{c}", name=f"wo{c}")
                nc.scalar.dma_start(out=t[:], in_=wo[128 * c:128 * (c + 1), :])
                wo_sb.append(t)
            tri_sb = pp.tile([128, 128], bf16, tag="tri", name="tri")
            nc.sync.dma_start(out=tri_sb[:], in_=tri[:, :])
            ones64_sb = pp.tile([1, 64], bf16, tag="ones64", name="ones64")
            nc.vector.memset(ones64_sb[:, :], 1.0)

            # persistent activation tensors; v tiles hold 65 cols per head
            # (64 v-channels + a ones column so PV also accumulates the
            # softmax denominator into output row 64)
            qT = [pp.tile([128, S], bf16, tag=f"qT{p}", name=f"qT{p}") for p in range(NP)]
            kT = [pp.tile([128, S], bf16, tag=f"kT{p}", name=f"kT{p}") for p in range(NP)]
            v_rm = [pp.tile([128, HPC * 65], bf16, tag=f"v{rt}", name=f"v{rt}") for rt in range(NRT)]
            oT = [pp.tile([128, S], bf16, tag=f"oT{p}", name=f"oT{p}") for p in range(NP)]

            def v_group_steps(rt):
                # V-projection psum group for row-tile rt, as one micro-step
                # (single matmul) per list entry so fillers can interleave
                # at ~200ns granularity between attention blocks
                state = {}

                def step(kt):
                    if kt == 0:
                        v_view = v_rm[rt].rearrange("p (h c) -> p h c", c=65)
                        nc.vector.memset(v_view[:, :, 64:65], 1.0)
                        state["ps"] = psmm.tile([128, 512], f32, tag="mm", name="mm")
                    nc.tensor.matmul(
                        state["ps"][:],
                        lhsT=xt_sb[kt][:, 128 * rt:128 * (rt + 1)],
                        rhs=wv_sb[kt][:],
                        start=(kt == 0),
                        stop=(kt == KT - 1),
                    )
                    if kt == KT - 1:
                        v_view = v_rm[rt].rearrange("p (h c) -> p h c", c=65)
                        nc.vector.tensor_copy(
                            v_view[:, :, 0:64],
                            state["ps"].rearrange("p (h c) -> p h c", c=64),
                        )

                return [lambda a=kt: step(a) for kt in range(KT)]

            def v_group(rt):
                for s in v_group_steps(rt):
                    s()

            def oproj_steps(rt):
                # out-projection row tile as 8 micro-steps + HBM store
                state = {}

                def step(i):
                    o2, c = i // 4, i % 4
                    if c == 0:
                        if o2 == 0:
                            state["st"] = ostagep.tile(
                                [128, 1024], f32, tag="ostage", name="ostage"
                            )
                        state["ps"] = psmm.tile([128, 512], f32, tag="mm", name="mm")
                    nc.tensor.matmul(
                        state["ps"][:],
                        lhsT=oT[c][:, 128 * rt:128 * (rt + 1)],
                        rhs=wo_sb[c][:, 512 * o2:512 * (o2 + 1)],
                        start=(c == 0),
                        stop=(c == 3),
                    )
                    if c == 3:
                        nc.vector.tensor_copy(
                            state["st"][:, 512 * o2:512 * (o2 + 1)], state["ps"][:]
                        )
                        if o2 == 1:
                            # store on the (idle) gpsimd queue so the big
                            # 512KB transfers never queue ahead of the
                            # latency-critical hi/normalize DMAs on sync
                            nc.gpsimd.dma_start(
                                out=out[128 * rt:128 * (rt + 1), :],
                                in_=state["st"][:],
                            )

                return [lambda a=i: step(a) for i in range(8)]

            def oproj_rt(rt):
                for s in oproj_steps(rt):
                    s()

            for _arep in range(rep_all):

             def qk_group_steps(p, ct, rc):
                 # one Q/K projection psum group as 8 micro-steps + evict
                 dst = qT[p] if ct < NP else kT[p]
                 state = {}

                 def step(kt):
                     if kt == 0:
                         state["ps"] = psmm.tile([128, 512], f32, tag="mm", name="mm")
                     nc.tensor.matmul(
                         state["ps"][:],
                         lhsT=wqk_sb[kt][:, 128 * ct:128 * (ct + 1)],
                         rhs=xt_sb[kt][:, 512 * rc:512 * (rc + 1)],
                         start=(kt == 0),
                         stop=(kt == KT - 1),
                     )
                     if kt == KT - 1:
                         nc.vector.tensor_copy(
                             dst[:, 512 * rc:512 * (rc + 1)], state["ps"][:]
                         )

                 return [lambda a=kt: step(a) for kt in range(KT)]

             def qk_steps(p):
                 return [
                     s
                     for ct in (p, NP + p)
                     for rc in range(NRC)
                     for s in qk_group_steps(p, ct, rc)
                 ]

             qk_groups = lambda p: [
                 (lambda a=p, b=ct, c=rc: [s() for s in qk_group_steps(a, b, c)])
                 for ct in (p, NP + p) for rc in range(NRC)
             ]

             def prologue(full=True):
                 # full prologue only for the first rep: for later reps,
                 # qk(0) is carried as filler inside the PREVIOUS rep's
                 # pairs 1-2 (legal: their WAR on qT[0]/kT[0] is released
                 # once that rep's pair 0 finishes), shrinking the serial
                 # PE-only region at the rep boundary to just V tiles 0-3.
                 if full:
                     for g in qk_groups(0):
                         g()
                 # V row-tiles 0..3 are needed by attention(0) st=0; the
                 # rest are interleave filler inside attention(0)
                 for rt in range(4):
                     v_group(rt)

             if _arep == 0:
                 prologue()
             carry_qk0 = _arep + 1 < rep_all

             def make_fill(p):
                 # per-st filler micro-step lists (one PE matmul each):
                 # sprinkled between every attention block so the PE always
                 # has work covering the exp latency. v_rm[rt] must be
                 # emitted before the st that streams it; qk(p+1) spreads
                 # over pair p; oproj(rt) spreads over pair 3 gated on the
                 # st that produced its oT rows.
                 fill = {0: [], 1: [], 2: [], 3: []}
                 if p == 0:
                     fill[0] = [s for rt in range(4, 8) for s in v_group_steps(rt)]
                     fill[1] = [s for rt in range(8, 16) for s in v_group_steps(rt)]
                     qs = qk_steps(1)
                     fill[2] = qs[: len(qs) // 2]
                     fill[3] = qs[len(qs) // 2:]
                 elif p < 3:
                     qs = qk_steps(p + 1)
                     fill[0] = qs[:8]
                     fill[1] = qs[8:24]
                     fill[2] = qs[24:44]
                     fill[3] = qs[44:]
                     if carry_qk0:
                         # next rep's qk(0) as extra filler for this
                         # under-filled exp-bound pair
                         q0 = qk_steps(0)
                         ex = q0[:32] if p == 1 else q0[32:]
                         fill[0] += ex[:4]
                         fill[1] += ex[4:12]
                         fill[2] += ex[12:21]
                         fill[3] += ex[21:]
                 else:
                     fill[1] = [s for rt in range(0, 4) for s in oproj_steps(rt)]
                     fill[2] = [s for rt in range(4, 8) for s in oproj_steps(rt)]
                     fill[3] = [s for rt in range(8, 12) for s in oproj_steps(rt)]
                 return fill

             pair_jobs = [(p, make_fill(p)) for p in range(NP)]
             for _extra in range(rep_attn - 1):
                 pair_jobs += [(p, {0: [], 1: [], 2: [], 3: []}) for p in range(NP)]

             for p, fill in pair_jobs:
                 hi_sb = hip.tile([64, S], bf16, tag="hi", name="hi")
                 for st in range(NST):
                     out_ps = [
                         psout.tile([65, 512], f32, tag="o", name="o")
                         for _ in range(2)
                     ]
                     nkb = 4 * st + 4
                     fillers = fill[st]
                     nfill = len(fillers)
                     fill_pos = 0

                     def pv_block(kb, pr):
                         r = kb - 4 * st
                         qi0 = 128 * r if r > 0 else 0
                         for hh in range(2):
                             base = 512 * hh
                             h = 2 * p + hh
                             nc.tensor.matmul(
                                 out_ps[hh][:, qi0:512],
                                 lhsT=v_rm[kb][:, 65 * h:65 * h + 65],
                                 rhs=pr[:, base + qi0:base + 512],
                                 start=(kb == 0),
                                 stop=(kb == nkb - 1),
                             )

                     # inner loop runs two blocks per unit, software-
                     # pipelined one unit deep: both blocks' scores+exp are
                     # emitted back-to-back (a 2-exp ACT backlog), then
                     # filler matmuls, then the previous unit's PVs — the
                     # PE streams projection work while ACT drains exps
                     # without the fillers postponing the score feed.
                     def sc_exp_block(kb):
                         r = kb - 4 * st  # >=0: diagonal block, tri-masked
                         qi0 = 128 * r if r > 0 else 0
                         sc = pssc.tile([128, 1024], f32, tag="sc", name="sc")
                         for hh in range(2):
                             base, lo = 512 * hh, 64 * hh
                             nc.tensor.matmul(
                                 sc[:, base + qi0:base + 512],
                                 lhsT=kT[p][lo:lo + 64, 128 * kb:128 * (kb + 1)],
                                 rhs=qT[p][lo:lo + 64, 512 * st + qi0:512 * (st + 1)],
                                 start=True,
                                 stop=True,
                                 tile_position=(lo, 0),
                             )
                         pr = probsp.tile([128, 1024], bf16, tag="pr", name="pr")
                         if qi0 == 0:
                             nc.scalar.activation(pr[:], sc[:], AF.Exp)
                         else:
                             pr_v = pr.rearrange("p (h q) -> p h q", h=2)
                             sc_v = sc.rearrange("p (h q) -> p h q", h=2)
                             nc.scalar.activation(
                                 pr_v[:, :, qi0:512], sc_v[:, :, qi0:512], AF.Exp
                             )
                         if r >= 0:
                             # causal zeroing of the diagonal stripe: probs
                             # *= upper-tri 0/1 mask (both heads, one DVE op)
                             pr_v = pr.rearrange("p (h q) -> p h q", h=2)
                             nc.vector.tensor_mul(
                                 pr_v[:, :, qi0:qi0 + 128],
                                 pr_v[:, :, qi0:qi0 + 128],
                                 tri_sb[:, None, :].broadcast_to([128, 2, 128]),
                             )
                         return (kb, pr)

                     prev_unit = []
                     for kb2 in range(0, nkb, 2):
                         cur = [sc_exp_block(kb2), sc_exp_block(kb2 + 1)]
                         target = nfill * (kb2 + 2) // nkb
                         while fill_pos < target:
                             fillers[fill_pos]()
                             fill_pos += 1
                         for item in prev_unit:
                             pv_block(*item)
                         prev_unit = cur
                     for item in prev_unit:
                         pv_block(*item)
                     # normalize: oT_dst = out_ps[0:64] * recip(denom row 64)
                     # with the recip row broadcast across partitions.
                     # hh=1 first so the partition-shift DMA issues while
                     # hh=0's recip/mul still run.
                     if NORM_MODE == "fast2":
                         # evict both [65,512] PSUM tiles immediately (data
                         # + denom row together — frees psout banks fast),
                         # then DMA-reshape the denom rows to [128,8] so
                         # the iterative reciprocal is ~100ns (cost scales
                         # with free size), reshape back, one gpsimd
                         # broadcast, and all-SBUF normalize multiplies.
                         o_sbs = []
                         for hh in (1, 0):
                             o_sb = recipp.tile(
                                 [65, 512], f32, tag=f"osb{hh}", name=f"osb{hh}"
                             )
                             nc.vector.tensor_copy(o_sb[:, :], out_ps[hh][:, :])
                             o_sbs.append((hh, o_sb))
                         d128 = recipp.tile([128, 8], f32, tag="d128", name="d128")
                         d128b = recipp.tile([128, 8], bf16, tag="d128b", name="d128b")
                         rrow = recipp.tile([1, 1024], bf16, tag="rc0", name="rc0")
                         bcast = recipp.tile([64, 1024], bf16, tag="bc", name="bc")
                         for hh, o_sb in o_sbs:
                             nc.sync.dma_start(
                                 out=d128[:, 4 * hh:4 * hh + 4],
                                 in_=o_sb[64:65, :],
                             )
                         with nc.allow_low_precision(reason="bf16 softmax denom"):
                             nc.vector.reciprocal(d128b[:, :], d128[:, :])
                         for hh, o_sb in o_sbs:
                             nc.sync.dma_start(
                                 out=rrow[0:1, 512 * hh:512 * (hh + 1)],
                                 in_=d128b[:, 4 * hh:4 * hh + 4],
                             )
                         nc.gpsimd.partition_broadcast(bcast[:, :], rrow[0:1, :])
                         for hh, o_sb in o_sbs:
                             dst = (
                                 oT[p][0:64, 512 * st:512 * (st + 1)]
                                 if hh == 0
                                 else hi_sb[:, 512 * st:512 * (st + 1)]
                             )
                             nc.vector.tensor_mul(
                                 dst,
                                 o_sb[0:64, :],
                                 bcast[:, 512 * hh:512 * (hh + 1)],
                             )
                             if hh == 1:
                                 # partition-shift DMA for the odd head
                                 nc.sync.dma_start(
                                     out=oT[p][64:128, 512 * st:512 * (st + 1)],
                                     in_=hi_sb[:, 512 * st:512 * (st + 1)],
                                 )
                     for hh in (1, 0) if NORM_MODE != "fast2" else ():
                         dst = (
                             oT[p][0:64, 512 * st:512 * (st + 1)]
                             if hh == 0
                             else hi_sb[:, 512 * st:512 * (st + 1)]
                         )
                         if True:
                             o_sb = recipp.tile([65, 512], f32, tag="osb", name="osb")
                             nc.vector.tensor_copy(o_sb[:, :], out_ps[hh][:, :])
                             rc_sb = recipp.tile([65, 512], bf16, tag="rc", name="rc")
                             rc0_sb = recipp.tile([1, 512], bf16, tag="rc0", name="rc0")
                             with nc.allow_low_precision(reason="bf16 softmax denom"):
                                 nc.vector.reciprocal(rc_sb[64:65, :], o_sb[64:65, :])
                             nc.sync.dma_start(out=rc0_sb[0:1, :], in_=rc_sb[64:65, :])
                             if NORM_MODE == "bcast":
                                 bcast = recipp.tile([64, 512], bf16, tag="bc", name="bc")
                                 nc.gpsimd.partition_broadcast(
                                     bcast[:, :], rc0_sb[0:1, :]
                                 )
                                 nc.vector.tensor_mul(
                                     dst, o_sb[0:64, :], bcast[:, :]
                                 )
                             else:
                                 bc_ps = psmm.tile([64, 512], f32, tag="mm", name="mm")
                                 nc.tensor.matmul(
                                     bc_ps[:, :],
                                     lhsT=ones64_sb[0:1, :],
                                     rhs=rc0_sb[0:1, :],
                                     start=True,
                                     stop=True,
                                 )
                                 nc.vector.tensor_mul(
                                     dst, o_sb[0:64, :], bc_ps[:, :]
                                 )
                         if hh == 1:
                             # partition-shift DMA for the odd head
                             nc.sync.dma_start(
                                 out=oT[p][64:128, 512 * st:512 * (st + 1)],
                                 in_=hi_sb[:, 512 * st:512 * (st + 1)],
                             )

             # phase-doubling knobs for differential HW attribution
             for _extra in range(rep_qkv - 1):
                 for p2 in range(NP):
                     for g in qk_groups(p2):
                         g()
                 for rt2 in range(NRT):
                     v_group(rt2)
             for _extra in range(rep_oproj - 1):
                 for rt2 in range(NRT):
                     oproj_rt(rt2)
             # software-pipeline across reps: the next rep's projection
             # prologue has no dependence on this rep's last normalize
             # chain, so emit it BEFORE the out-projection tail — the PE
             # FIFO then fills the tail's dependency wait with real work.
             if _arep + 1 < rep_all:
                 prologue(full=False)
             # ---- out-projection tail: last four row tiles ----
             for rt in range(12, 16):
                 oproj_rt(rt)
    nc.finalize()
    return nc


def get_program():
    if "nc" not in _PROG_CACHE:
        _PROG_CACHE["nc"] = build_program()
    return _PROG_CACHE["nc"]


def make_in_maps(x, w_qkv, w_out):
    import ml_dtypes

    bf = ml_dtypes.bfloat16
    x = np.asarray(x, dtype=np.float32)
    w_qkv = np.asarray(w_qkv, dtype=np.float32)
    w_out = np.asarray(w_out, dtype=np.float32)
    scale = float(D) ** -0.5
    # upper-tri (incl diagonal) 0/1 mask: element [p, j] of a diagonal
    # 128x128 stripe survives iff key p <= query j
    p_idx = np.arange(128)[:, None]
    j_idx = np.arange(128)[None, :]
    tri = (p_idx <= j_idx).astype(bf)
    in_maps = []
    for c in range(NCORES):
        b, hh = c // 2, c % 2
        q0 = CQ * hh
        wq = (w_qkv[:, q0:q0 + CQ] * scale).astype(bf)
        wk = w_qkv[:, DM + q0:DM + q0 + CQ].astype(bf)
        in_maps.append(
            {
                "xt": np.ascontiguousarray(x[b].T).astype(bf),
                "wqk": np.concatenate([wq, wk], axis=1),
                "wv": w_qkv[:, 2 * DM + q0:2 * DM + q0 + CQ].astype(bf),
                "wo": w_out[q0:q0 + CQ, :].astype(bf),
                "tri": tri,
            }
        )
    return in_maps


def gather(results):
    outs = [np.asarray(results[c]["out"], dtype=np.float32) for c in range(NCORES)]
    return np.stack([outs[2 * b] + outs[2 * b + 1] for b in range(B)], axis=0)


def kernel(x, w_qkv, w_out):
    from concourse.bass_utils import run_bass_kernel_spmd

    nc = get_program()
    in_maps = make_in_maps(x, w_qkv, w_out)
    res = run_bass_kernel_spmd(nc, in_maps, list(range(NCORES)))
    return gather(res.results)
